# revision 11
# baseline (speedup 1.0000x reference)
"""Trainium2 Bass kernel for nn_ContextAugHead (segment_reduce).

Full-input contract: kernel(**inputs) takes the unsharded inputs of
reference.setup_inputs() and returns the same nested structure as
reference.reference(): ((sep_loss, sep_result), (tok_loss, tok_result)).

Strategy (data-parallel over batch, one NeuronCore per batch row):
  cell_states = segment-average over tokens with equal `indicator` value
  (<=256 distinct values), so per batch:
    segT[H,V]   = X^T @ onehot(ind)          (PE, contraction over tokens)
    A_v[V,6]    = onehot(ind)^T @ L6         (per-(segment,label,parity) histogram)
    seg_avgT    = segT * (1/max(counts,1))   broadcast over H
    hT          = tanh(WuT^T @ seg_avgT + bu)
    logitsT     = WpT^T @ hT + bp            ([3,V])
  then per-segment log-softmax / argmax on [V,3] strips, and the final
  loss numerators / metric counts reduce through one tiny matmul:
    M2[6,8] = A_v^T @ [P_onehot | ones | -nll | 0]
  Host sums M2 over the 8 cores (the "all-reduce") and finishes the
  scalar divisions (loss, precision/recall/f1).

Matmul precision modes (per stage): 'hilo' = 2x/3x bf16 hi+lo passes
(numerically ~fp32); 'f32r' = single-pass float32r (tf32-class).
"""
import sys
import types
import ctypes
import contextlib

sys.path.insert(0, "/opt/trn_rl_repo")

import numpy as np
import ml_dtypes

import concourse.bass as bass
import concourse.bacc as bacc
import concourse.tile as tile
from concourse import mybir
from concourse.masks import make_identity
from concourse.bass_utils import run_bass_kernel_spmd

F32 = mybir.dt.float32
F32R = mybir.dt.float32r
BF16 = mybir.dt.bfloat16
I32 = mybir.dt.int32
AF = mybir.ActivationFunctionType
OP = mybir.AluOpType

B, S, H, NCLS, V = 8, 2048, 768, 3, 256
KT = S // 128          # 16 token tiles
HJ = H // 128          # 6 H tiles
NG = 4                 # token tiles per DMA group
NGRP = KT // NG

SEG_MODE = "f32r"      # 'hilo' | 'f32r'   (X^T @ onehot)
HEAD_MODE = "f32r"     # 'hilo' | 'f32r'   (Wu/Wp matmuls)


def _install_ntff_hook():
    """The image's antenv lacks axon_hooks; register the NTFF profile hook
    (used only when tracing) via ctypes against libaxon_pjrt.so."""
    if "antenv.axon_hooks" in sys.modules:
        return
    try:
        lib = ctypes.CDLL("/opt/axon/libaxon_pjrt.so")
        if not hasattr(lib, "axon_start_nrt_profile"):
            return
    except OSError:
        return
    lib.axon_start_nrt_profile.argtypes = [ctypes.POINTER(ctypes.c_int64),
                                           ctypes.c_size_t]
    lib.axon_start_nrt_profile.restype = ctypes.c_int64
    lib.axon_stop_nrt_profile.argtypes = [ctypes.c_char_p]
    lib.axon_stop_nrt_profile.restype = ctypes.c_int64

    @contextlib.contextmanager
    def _hook(output_dir, device_ids):
        import jax
        jax.devices()
        if device_ids:
            ids = (ctypes.c_int64 * len(device_ids))(*device_ids)
            rc = lib.axon_start_nrt_profile(ids, len(device_ids))
        else:
            rc = lib.axon_start_nrt_profile(None, 0)
        if rc != 0:
            raise RuntimeError(f"axon_start_nrt_profile rc={rc}")
        try:
            yield
        finally:
            n = lib.axon_stop_nrt_profile(str(output_dir).encode())
            print(f"[ntff] {n} profile file(s) -> {output_dir}", file=sys.stderr)

    mod = types.ModuleType("antenv.axon_hooks")
    mod.get_axon_ntff_profile_hook = lambda: _hook
    mod.set_axon_ntff_profile_hook = lambda h: None
    sys.modules["antenv.axon_hooks"] = mod


_install_ntff_hook()


def build_nc(seg_mode=SEG_MODE, head_mode=HEAD_MODE):
    nc = bacc.Bacc(trn_type="TRN2", target_bir_lowering=False, debug=False,
                   num_devices=8)

    # ---------------- DRAM I/O ----------------
    if seg_mode == "hilo":
        xh_d = nc.dram_tensor("xh", [KT, 128, H], BF16, kind="ExternalInput")
        xl_d = nc.dram_tensor("xl", [KT, 128, H], BF16, kind="ExternalInput")
    else:
        xf_d = nc.dram_tensor("xf", [KT, 128, H], F32R, kind="ExternalInput")
    ind_d = nc.dram_tensor("indt", [128, KT], F32, kind="ExternalInput")
    lab_d = nc.dram_tensor("labt", [128, KT], F32, kind="ExternalInput")
    par_d = nc.dram_tensor("par6", [128, 6], BF16, kind="ExternalInput")
    if head_mode == "hilo":
        wuh_d = nc.dram_tensor("wut_h", [128, HJ, H], BF16, kind="ExternalInput")
        wul_d = nc.dram_tensor("wut_l", [128, HJ, H], BF16, kind="ExternalInput")
        wph_d = nc.dram_tensor("wpt_h", [128, HJ, NCLS], BF16, kind="ExternalInput")
        wpl_d = nc.dram_tensor("wpt_l", [128, HJ, NCLS], BF16, kind="ExternalInput")
    else:
        wuf_d = nc.dram_tensor("wut_f", [128, HJ, H], F32R, kind="ExternalInput")
        wpf_d = nc.dram_tensor("wpt_f", [128, HJ, NCLS], F32R, kind="ExternalInput")
    bu_d = nc.dram_tensor("but", [128, HJ], F32, kind="ExternalInput")
    bp_d = nc.dram_tensor("bp3", [NCLS, 1], F32, kind="ExternalInput")
    m2_d = nc.dram_tensor("m2", [6, 8], F32, kind="ExternalOutput")

    with tile.TileContext(nc) as tc:
        with (
            tc.tile_pool(name="consts", bufs=1) as consts,
            tc.tile_pool(name="xin", bufs=2) as xin,
            tc.tile_pool(name="oh", bufs=4) as ohp,
            tc.tile_pool(name="head", bufs=1) as headp,
            tc.tile_pool(name="fin", bufs=1) as finp,
        ):
            # --------- small consts + index DMAs (issue first, tiny) ---------
            ind_sb = consts.tile([128, KT], F32)
            nc.gpsimd.dma_start(ind_sb, ind_d[:, :])
            lab_sb = consts.tile([128, KT], F32)
            nc.gpsimd.dma_start(lab_sb, lab_d[:, :])
            par_sb = consts.tile([128, 6], BF16)
            nc.gpsimd.dma_start(par_sb, par_d[:, :])
            bu_sb = consts.tile([128, HJ], F32)
            nc.gpsimd.dma_start(bu_sb, bu_d[:, :])
            bp_sb = consts.tile([NCLS, 1], F32)
            nc.gpsimd.dma_start(bp_sb, bp_d[:, :])

            iota_i = consts.tile([128, V], I32)
            nc.gpsimd.iota(iota_i, pattern=[[1, V]], base=0, channel_multiplier=0)
            iota_f = consts.tile([128, V], BF16)
            nc.vector.tensor_copy(iota_f, iota_i)
            iota6_i = consts.tile([128, 6], I32)
            nc.gpsimd.iota(iota6_i, pattern=[[0, 2], [1, 3]], base=0,
                           channel_multiplier=0)
            iota6_f = consts.tile([128, 6], BF16)
            nc.vector.tensor_copy(iota6_f, iota6_i)
            ident = consts.tile([128, 128], F32)
            make_identity(nc, ident)

            # --------- grouped X DMAs (hi on Sync queue, lo on GPSIMD) ---------
            if seg_mode == "hilo":
                xh_g = [xin.tile([128, NG, H], BF16, tag=f"xh{g%2}", name=f"xhg{g}")
                        for g in range(NGRP)]
                xl_g = [xin.tile([128, NG, H], BF16, tag=f"xl{g%2}", name=f"xlg{g}")
                        for g in range(NGRP)]
                for g in range(NGRP):
                    nc.sync.dma_start(
                        xh_g[g], xh_d[g * NG:(g + 1) * NG, :, :]
                        .rearrange("k p h -> p k h"))
                    nc.sync.dma_start(
                        xl_g[g], xl_d[g * NG:(g + 1) * NG, :, :]
                        .rearrange("k p h -> p k h"))
            else:
                xf_g = [xin.tile([128, NG, H], F32R, tag=f"xf{g%2}", name=f"xfg{g}")
                        for g in range(NGRP)]
                for g in range(NGRP):
                    nc.sync.dma_start(
                        xf_g[g], xf_d[g * NG:(g + 1) * NG, :, :]
                        .rearrange("k p h -> p k h"))

            # --------- weight DMAs (Scalar HWDGE queue, after x issues) ---------
            if head_mode == "hilo":
                wuh_sb = consts.tile([128, HJ, H], BF16)
                wul_sb = consts.tile([128, HJ, H], BF16)
                wph_sb = consts.tile([128, HJ, NCLS], BF16)
                wpl_sb = consts.tile([128, HJ, NCLS], BF16)
                nc.scalar.dma_start(wuh_sb, wuh_d[:, :, :])
                nc.scalar.dma_start(wul_sb, wul_d[:, :, :])
                nc.scalar.dma_start(wph_sb, wph_d[:, :, :])
                nc.scalar.dma_start(wpl_sb, wpl_d[:, :, :])
            else:
                wuf_sb = consts.tile([128, HJ, H], F32R)
                wpf_sb = consts.tile([128, HJ, NCLS], F32R)
                nc.scalar.dma_start(wuf_sb, wuf_d[:, :, :])
                nc.scalar.dma_start(wpf_sb, wpf_d[:, :, :])

            # ---------------- phase 1: token loop ----------------
            with tc.tile_pool(name="ps1", bufs=1, space="PSUM") as ps1:
                ps_seg = [ps1.tile([128, V], F32, tag=f"seg{m}", name=f"seg{m}")
                          for m in range(HJ)]
                ps_A = [ps1.tile([128, 6], F32, tag=f"av{s}", name=f"av{s}")
                        for s in range(2)]

                o_dt = BF16 if seg_mode == "hilo" else F32R
                # one-hots + histogram matmuls first: A_v (and the 1/counts
                # broadcast chain) completes while the seg matmuls still run
                O_t = [ohp.tile([128, V], o_dt, tag="O", name=f"O_{k}", bufs=KT)
                       for k in range(KT)]
                for k in range(KT):
                    nc.vector.tensor_scalar(
                        out=O_t[k], in0=iota_f, scalar1=ind_sb[:, k:k + 1],
                        scalar2=None, op0=OP.is_equal)
                    # L6[p,c'] = (lab==c'%3) * parity_mask[p,c']
                    l3_k = ohp.tile([128, 6], BF16, tag="l3", name="l3_k")
                    nc.vector.tensor_scalar(
                        out=l3_k, in0=iota6_f, scalar1=lab_sb[:, k:k + 1],
                        scalar2=None, op0=OP.is_equal)
                    l6_k = ohp.tile([128, 6], o_dt, tag="l6", name="l6_k")
                    nc.vector.tensor_tensor(out=l6_k, in0=l3_k, in1=par_sb,
                                            op=OP.mult)
                    for s in range(2):
                        nc.tensor.matmul(ps_A[s], lhsT=O_t[k][:, s * 128:(s + 1) * 128],
                                         rhs=l6_k, start=(k == 0), stop=(k == KT - 1))

                # -------- stats chain (overlaps the seg loop below) --------
                A_v = [finp.tile([128, 6], F32, tag=f"avsb{s}", name=f"avsb{s}")
                       for s in range(2)]
                recip = [finp.tile([128, 1], F32, tag=f"rc{s}", name=f"rc{s}")
                         for s in range(2)]
                for s in range(2):
                    cnt = finp.tile([128, 1], F32, tag=f"cnt{s}", name=f"cnt{s}")
                    # ACT copy evacuates PSUM and row-sums in one op
                    nc.scalar.activation(A_v[s], ps_A[s], AF.Copy, accum_out=cnt)
                    nc.vector.tensor_scalar_max(cnt, cnt, 1.0)
                    nc.vector.reciprocal(recip[s], cnt)

                # broadcast 1/counts to [128, V] via transpose + K=1 matmul
                # (reuse the released histogram banks av0/av1 of ps1)
                r_row = finp.tile([1, V], F32, tag="rrow")
                for s in range(2):
                    ps_t = ps1.tile([128, 128], F32, tag="av0", name="ps_t")
                    nc.tensor.transpose(ps_t[0:1, 0:128], recip[s], ident)
                    nc.vector.tensor_copy(r_row[0:1, s * 128:(s + 1) * 128],
                                          ps_t[0:1, 0:128])
                ones_row = consts.tile([1, 128], F32)
                nc.vector.memset(ones_row, 1.0)
                ps_bc = ps1.tile([128, V], F32, tag="av1", name="ps_bc")
                nc.tensor.matmul(ps_bc, lhsT=ones_row, rhs=r_row,
                                 start=True, stop=True)
                bc_sb = finp.tile([128, V], F32, tag="bc")
                nc.vector.tensor_copy(bc_sb, ps_bc)

                # -------- seg matmuls --------
                for k in range(KT):
                    g, kk = k // NG, k % NG
                    if seg_mode == "hilo":
                        for m in range(HJ):
                            nc.tensor.matmul(ps_seg[m],
                                             lhsT=xh_g[g][:, kk, m * 128:(m + 1) * 128],
                                             rhs=O_t[k], start=(k == 0), stop=False)
                        for m in range(HJ):
                            nc.tensor.matmul(ps_seg[m],
                                             lhsT=xl_g[g][:, kk, m * 128:(m + 1) * 128],
                                             rhs=O_t[k], start=False,
                                             stop=(k == KT - 1))
                    else:
                        for m in range(HJ):
                            nc.tensor.matmul(ps_seg[m],
                                             lhsT=xf_g[g][:, kk, m * 128:(m + 1) * 128],
                                             rhs=O_t[k],
                                             start=(k == 0), stop=(k == KT - 1))

                # seg_avgT tiles (+ hi/lo split if needed)
                sa_dt = F32 if head_mode == "hilo" else F32R
                sa_f = [headp.tile([128, V], sa_dt, tag=f"saf{m}", name=f"saf{m}")
                        for m in range(HJ)]
                if head_mode == "hilo":
                    sa_h = [headp.tile([128, V], BF16, tag=f"sah{m}", name=f"sah{m}")
                            for m in range(HJ)]
                    sa_l = [headp.tile([128, V], BF16, tag=f"sal{m}", name=f"sal{m}")
                            for m in range(HJ)]
                for m in range(HJ):
                    nc.vector.tensor_tensor(out=sa_f[m], in0=ps_seg[m],
                                            in1=bc_sb, op=OP.mult)
                    if head_mode == "hilo":
                        nc.vector.tensor_copy(sa_h[m], sa_f[m])
                        nc.vector.tensor_tensor(out=sa_l[m], in0=sa_f[m],
                                                in1=sa_h[m], op=OP.subtract)

            # ---------------- phase 3: head ----------------
            th_dt = F32 if head_mode == "hilo" else F32R
            th = [headp.tile([128, V], th_dt, tag=f"th{m}", name=f"th{m}")
                  for m in range(HJ)]
            if head_mode == "hilo":
                hh = [headp.tile([128, V], BF16, tag=f"hh{m}", name=f"hh{m}")
                      for m in range(HJ)]
                hl = [headp.tile([128, V], BF16, tag=f"hl{m}", name=f"hl{m}")
                      for m in range(HJ)]
            with tc.tile_pool(name="ps3", bufs=1, space="PSUM") as ps3:
                for j in range(HJ):
                    ps_h = ps3.tile([128, V], F32, tag="psh", name="ps_h", bufs=2)
                    # kk-outer so PE consumption pipelines with the DVE splits
                    if head_mode == "hilo":
                        n_mm, i_mm = 3 * HJ, 0
                        for kk in range(HJ):
                            for w_sb, x_t in ((wuh_sb, sa_h[kk]), (wuh_sb, sa_l[kk]),
                                              (wul_sb, sa_h[kk])):
                                nc.tensor.matmul(
                                    ps_h, lhsT=w_sb[:, kk, j * 128:(j + 1) * 128],
                                    rhs=x_t, start=(i_mm == 0),
                                    stop=(i_mm == n_mm - 1))
                                i_mm += 1
                    else:
                        for kk in range(HJ):
                            nc.tensor.matmul(
                                ps_h, lhsT=wuf_sb[:, kk, j * 128:(j + 1) * 128],
                                rhs=sa_f[kk], start=(kk == 0),
                                stop=(kk == HJ - 1))
                    # tanh(pre + bu) from PSUM -> SBUF
                    nc.scalar.activation(th[j], ps_h, AF.Tanh,
                                         bias=bu_sb[:, j:j + 1], scale=1.0)
                    if head_mode == "hilo":
                        nc.vector.tensor_copy(hh[j], th[j])
                        nc.vector.tensor_tensor(out=hl[j], in0=th[j], in1=hh[j],
                                                op=OP.subtract)

                # logits
                ps_l = ps3.tile([NCLS, V], F32, tag="psl", name="ps_l")
                if head_mode == "hilo":
                    n_mm, i_mm = 3 * HJ, 0
                    for kk in range(HJ):
                        for w_sb, x_t in ((wph_sb, hh[kk]), (wph_sb, hl[kk]),
                                          (wpl_sb, hh[kk])):
                            nc.tensor.matmul(ps_l, lhsT=w_sb[:, kk, :], rhs=x_t,
                                             start=(i_mm == 0),
                                             stop=(i_mm == n_mm - 1))
                            i_mm += 1
                else:
                    for kk in range(HJ):
                        nc.tensor.matmul(ps_l, lhsT=wpf_sb[:, kk, :],
                                         rhs=th[kk],
                                         start=(kk == 0), stop=(kk == HJ - 1))
                l_sb = finp.tile([NCLS, V], F32, tag="lsb")
                nc.vector.tensor_scalar(out=l_sb, in0=ps_l, scalar1=bp_sb[:, 0:1],
                                        scalar2=None, op0=OP.add)

                # ------------- phase 4: per-segment softmax/argmax -------------
                lv = [finp.tile([128, NCLS], F32, tag=f"lv{s}", name=f"lv{s}")
                      for s in range(2)]
                for s in range(2):
                    ps_t2 = ps3.tile([128, NCLS], F32, tag="pst2", name="ps_t2")
                    nc.tensor.transpose(ps_t2[0:128, 0:NCLS],
                                        l_sb[:, s * 128:(s + 1) * 128],
                                        ident[0:NCLS, 0:NCLS])
                    nc.vector.tensor_copy(lv[s], ps_t2)

                R = [finp.tile([128, 8], F32, tag=f"R{s}", name=f"R{s}")
                     for s in range(2)]
                for s in range(2):
                    ve = nc.vector
                    nc.gpsimd.memset(R[s], 0.0)
                    mneg = finp.tile([128, 1], F32, tag=f"mn{s}", name=f"mn{s}")
                    nc.vector.tensor_reduce(mneg, lv[s],
                                            axis=mybir.AxisListType.X,
                                            op=OP.max, negate=True)
                    e_t = finp.tile([128, NCLS], F32, tag=f"et{s}", name=f"et{s}")
                    ssum = finp.tile([128, 1], F32, tag=f"ss{s}", name=f"ss{s}")
                    nc.scalar.activation(e_t, lv[s], AF.Exp, bias=mneg, scale=1.0,
                                         accum_out=ssum)
                    lse = finp.tile([128, 1], F32, tag=f"lse{s}", name=f"lse{s}")
                    nc.scalar.activation(lse, ssum, AF.Ln)
                    # lse := ln(sum) - mneg = logsumexp
                    ve.tensor_tensor(out=lse, in0=lse, in1=mneg, op=OP.subtract)
                    # cols 4:7 = l - lse = -nll
                    ve.tensor_scalar(out=R[s][:, 4:7], in0=lv[s],
                                     scalar1=lse, scalar2=None, op0=OP.subtract)
                    # eq = (l + mneg) >= 0  (one-hot of rowmax, ties -> multiple)
                    eq = finp.tile([128, NCLS], F32, tag=f"eq{s}", name=f"eq{s}")
                    ve.tensor_scalar(out=eq, in0=lv[s], scalar1=mneg,
                                     scalar2=0.0, op0=OP.add, op1=OP.is_ge)
                    # first-argmax tie-break into cols 0:3
                    ve.tensor_copy(R[s][:, 0:1], eq[:, 0:1])
                    om0 = finp.tile([128, 1], F32, tag=f"om0{s}", name=f"om0{s}")
                    ve.tensor_scalar(out=om0, in0=eq[:, 0:1], scalar1=-1.0,
                                     scalar2=1.0, op0=OP.mult, op1=OP.add)
                    ve.tensor_tensor(out=R[s][:, 1:2], in0=eq[:, 1:2],
                                     in1=om0, op=OP.mult)
                    om1 = finp.tile([128, 1], F32, tag=f"om1{s}", name=f"om1{s}")
                    ve.tensor_scalar(out=om1, in0=eq[:, 1:2], scalar1=-1.0,
                                     scalar2=1.0, op0=OP.mult, op1=OP.add)
                    ve.tensor_tensor(out=om1, in0=om0, in1=om1, op=OP.mult)
                    ve.tensor_tensor(out=R[s][:, 2:3], in0=eq[:, 2:3],
                                     in1=om1, op=OP.mult)
                    nc.gpsimd.memset(R[s][:, 3:4], 1.0)

                # ------------- M2 = A_v^T @ R -------------
                ps_m = ps3.tile([6, 8], F32, tag="psm", name="ps_m")
                for s in range(2):
                    nc.tensor.matmul(ps_m, lhsT=A_v[s], rhs=R[s][:, 0:8],
                                     start=(s == 0), stop=(s == 1))
                m2_sb = finp.tile([6, 8], F32, tag="m2sb")
                nc.vector.tensor_copy(m2_sb, ps_m)
                nc.scalar.dma_start(m2_d[:, :], m2_sb)

    nc.compile()
    return nc


# ---------------------------------------------------------------------------
# host side
# ---------------------------------------------------------------------------
_NC_CACHE = {}


def _get_nc(seg_mode=SEG_MODE, head_mode=HEAD_MODE):
    key = (seg_mode, head_mode)
    if key not in _NC_CACHE:
        _NC_CACHE[key] = build_nc(*key)
    return _NC_CACHE[key]


def _split_hilo(x):
    hi = x.astype(ml_dtypes.bfloat16)
    lo = (x - hi.astype(np.float32)).astype(ml_dtypes.bfloat16)
    return hi, lo


def make_in_maps(encoded_states, indicator, ca_label, Wu, bu, Wp, bp,
                 seg_mode=SEG_MODE, head_mode=HEAD_MODE):
    encoded_states = np.asarray(encoded_states, dtype=np.float32)
    indicator = np.asarray(indicator)
    ca_label = np.asarray(ca_label)
    Wu = np.asarray(Wu, dtype=np.float32)
    Wp = np.asarray(Wp, dtype=np.float32)
    bu = np.asarray(bu, dtype=np.float32)
    bp = np.asarray(bp, dtype=np.float32)

    # shared (weight) tensors
    # SBUF layout [128, HJ, H]: partition p holds WuT[j*128+p, :] per j
    WuT = np.ascontiguousarray(
        Wu.T.reshape(HJ, 128, H).transpose(1, 0, 2))           # [128, HJ, H]
    WpT = np.ascontiguousarray(
        Wp.T.reshape(HJ, 128, NCLS).transpose(1, 0, 2))
    but = np.ascontiguousarray(bu.reshape(HJ, 128).T)          # [128, HJ]
    bp3 = bp.reshape(NCLS, 1)
    par6 = np.zeros((128, 6), ml_dtypes.bfloat16)
    par6[0::2, 0:3] = 1.0
    par6[1::2, 3:6] = 1.0

    shared = {"but": but, "bp3": bp3, "par6": par6}
    if head_mode == "hilo":
        wuh, wul = _split_hilo(WuT)
        wph, wpl = _split_hilo(WpT)
        shared.update({"wut_h": wuh, "wut_l": wul, "wpt_h": wph, "wpt_l": wpl})
    else:
        shared.update({"wut_f": WuT, "wpt_f": WpT})

    in_maps = []
    for b in range(B):
        x = encoded_states[b].reshape(KT, 128, H)
        ind = np.ascontiguousarray(
            indicator[b].astype(np.float32).reshape(KT, 128).T)  # [128,KT]
        lab = np.ascontiguousarray(
            ca_label[b].astype(np.float32).reshape(KT, 128).T)
        m = {"indt": ind, "labt": lab, **shared}
        if seg_mode == "hilo":
            xh, xl = _split_hilo(x)
            m.update({"xh": xh, "xl": xl})
        else:
            m.update({"xf": x})
        in_maps.append(m)
    return in_maps


def assemble(m2_list, loss_weight):
    """Sum per-core M2 [6,8] and finish the scalars exactly as reference."""
    w = np.asarray(loss_weight, dtype=np.float32)
    M = np.zeros((6, 8), np.float64)
    for m2 in m2_list:
        M += m2.astype(np.float64)
    out = []
    for h in range(2):
        Mh = M[3 * h:3 * h + 3, :]
        num = -np.float32(sum(np.float64(w[c]) * Mh[c, 4 + c] for c in range(3)))
        den = np.float32(sum(np.float64(w[c]) * Mh[c, 3] for c in range(3)))
        loss = np.float32(num / den)
        res = {}
        for t in range(3):
            tp = np.int32(round(Mh[t, t]))
            fp = np.int32(round(Mh[:, t].sum() - Mh[t, t]))
            fn = np.int32(round(Mh[t, 3] - Mh[t, t]))
            precision = np.float32(tp) / np.float32(tp + fp)
            recall = np.float32(tp) / np.float32(tp + fn)
            f1 = np.float32(2) * precision * recall / (precision + recall)
            res[t] = {'precision': np.float32(precision),
                      'recall': np.float32(recall), 'f1': np.float32(f1),
                      'tp': tp, 'fp': fp, 'fn': fn}
        out.append((loss, res))
    return ((out[0][0], out[0][1]), (out[1][0], out[1][1]))


def run_device(inputs, seg_mode=SEG_MODE, head_mode=HEAD_MODE, trace=False,
               trace_cores=None):
    nc = _get_nc(seg_mode, head_mode)
    in_maps = make_in_maps(
        inputs["encoded_states"], inputs["indicator"], inputs["ca_label"],
        inputs["Wu"], inputs["bu"], inputs["Wp"], inputs["bp"],
        seg_mode=seg_mode, head_mode=head_mode)
    res = run_bass_kernel_spmd(nc, in_maps, core_ids=list(range(B)),
                               trace=trace, trace_cores=trace_cores)
    m2s = [res.results[c]["m2"] for c in range(B)]
    return m2s, res


def kernel(encoded_states, indicator, ca_label, Wu, bu, Wp, bp, loss_weight):
    inputs = {"encoded_states": encoded_states, "indicator": indicator,
              "ca_label": ca_label, "Wu": Wu, "bu": bu, "Wp": Wp, "bp": bp}
    m2s, _ = run_device(inputs)
    return assemble(m2s, loss_weight)


# revision 12
# speedup vs baseline: 1.1623x; 1.1623x over previous
"""Trainium2 Bass kernel for nn_ContextAugHead (segment_reduce).

Full-input contract: kernel(**inputs) takes the unsharded inputs of
reference.setup_inputs() and returns the same nested structure as
reference.reference(): ((sep_loss, sep_result), (tok_loss, tok_result)).

Strategy (data-parallel over batch, one NeuronCore per batch row):
  cell_states = segment-average over tokens with equal `indicator` value
  (<=256 distinct values), so per batch:
    segT[H,V]   = X^T @ onehot(ind)          (PE, contraction over tokens)
    A_v[V,6]    = onehot(ind)^T @ L6         (per-(segment,label,parity) histogram)
    seg_avgT    = segT * (1/max(counts,1))   broadcast over H
    hT          = tanh(WuT^T @ seg_avgT + bu)
    logitsT     = WpT^T @ hT + bp            ([3,V])
  then per-segment log-softmax / argmax on [V,3] strips, and the final
  loss numerators / metric counts reduce through one tiny matmul:
    M2[6,8] = A_v^T @ [P_onehot | ones | -nll | 0]
  Host sums M2 over the 8 cores (the "all-reduce") and finishes the
  scalar divisions (loss, precision/recall/f1).

Matmul precision modes (per stage): 'hilo' = 2x/3x bf16 hi+lo passes
(numerically ~fp32); 'f32r' = single-pass float32r (tf32-class).
"""
import sys
import types
import ctypes
import contextlib

sys.path.insert(0, "/opt/trn_rl_repo")

import numpy as np
import ml_dtypes

import concourse.bass as bass
import concourse.bacc as bacc
import concourse.tile as tile
from concourse import mybir
from concourse.masks import make_identity
from concourse.bass_utils import run_bass_kernel_spmd

F32 = mybir.dt.float32
F32R = mybir.dt.float32r
BF16 = mybir.dt.bfloat16
I32 = mybir.dt.int32
AF = mybir.ActivationFunctionType
OP = mybir.AluOpType

B, S, H, NCLS, V = 8, 2048, 768, 3, 256
KT = S // 128          # 16 token tiles
HJ = H // 128          # 6 H tiles
NG = 4                 # token tiles per DMA group
NGRP = KT // NG

SEG_MODE = "f32r"      # 'hilo' | 'f32r'   (X^T @ onehot)
HEAD_MODE = "f32r"     # 'hilo' | 'f32r'   (Wu/Wp matmuls)


def _install_ntff_hook():
    """The image's antenv lacks axon_hooks; register the NTFF profile hook
    (used only when tracing) via ctypes against libaxon_pjrt.so."""
    if "antenv.axon_hooks" in sys.modules:
        return
    try:
        lib = ctypes.CDLL("/opt/axon/libaxon_pjrt.so")
        if not hasattr(lib, "axon_start_nrt_profile"):
            return
    except OSError:
        return
    lib.axon_start_nrt_profile.argtypes = [ctypes.POINTER(ctypes.c_int64),
                                           ctypes.c_size_t]
    lib.axon_start_nrt_profile.restype = ctypes.c_int64
    lib.axon_stop_nrt_profile.argtypes = [ctypes.c_char_p]
    lib.axon_stop_nrt_profile.restype = ctypes.c_int64

    @contextlib.contextmanager
    def _hook(output_dir, device_ids):
        import jax
        jax.devices()
        if device_ids:
            ids = (ctypes.c_int64 * len(device_ids))(*device_ids)
            rc = lib.axon_start_nrt_profile(ids, len(device_ids))
        else:
            rc = lib.axon_start_nrt_profile(None, 0)
        if rc != 0:
            raise RuntimeError(f"axon_start_nrt_profile rc={rc}")
        try:
            yield
        finally:
            n = lib.axon_stop_nrt_profile(str(output_dir).encode())
            print(f"[ntff] {n} profile file(s) -> {output_dir}", file=sys.stderr)

    mod = types.ModuleType("antenv.axon_hooks")
    mod.get_axon_ntff_profile_hook = lambda: _hook
    mod.set_axon_ntff_profile_hook = lambda h: None
    sys.modules["antenv.axon_hooks"] = mod


_install_ntff_hook()


def build_nc(seg_mode=SEG_MODE, head_mode=HEAD_MODE):
    nc = bacc.Bacc(trn_type="TRN2", target_bir_lowering=False, debug=False,
                   num_devices=8)

    # ---------------- DRAM I/O ----------------
    if seg_mode == "hilo":
        xh_d = nc.dram_tensor("xh", [KT, 128, H], BF16, kind="ExternalInput")
        xl_d = nc.dram_tensor("xl", [KT, 128, H], BF16, kind="ExternalInput")
    else:
        xf_d = nc.dram_tensor("xf", [KT, 128, H], F32R, kind="ExternalInput")
    # misc pack: cols [0:KT]=indicator, [KT:2KT]=labels, [2KT:2KT+6]=parity6
    misc_d = nc.dram_tensor("miscp", [128, 2 * KT + 6], F32, kind="ExternalInput")
    if head_mode == "hilo":
        wuh_d = nc.dram_tensor("wut_h", [128, HJ, H], BF16, kind="ExternalInput")
        wul_d = nc.dram_tensor("wut_l", [128, HJ, H], BF16, kind="ExternalInput")
        wph_d = nc.dram_tensor("wpt_h", [128, HJ, NCLS], BF16, kind="ExternalInput")
        wpl_d = nc.dram_tensor("wpt_l", [128, HJ, NCLS], BF16, kind="ExternalInput")
    else:
        wuf_d = nc.dram_tensor("wut_f", [128, HJ, H], F32R, kind="ExternalInput")
        wpf_d = nc.dram_tensor("wpt_f", [128, HJ, NCLS], F32R, kind="ExternalInput")
    # bias pack: cols [0:HJ]=bu tiles, col HJ rows 0:3 = bp
    bias_d = nc.dram_tensor("biasp", [128, HJ + 1], F32, kind="ExternalInput")
    m2_d = nc.dram_tensor("m2", [6, 8], F32, kind="ExternalOutput")

    with tile.TileContext(nc) as tc:
        with (
            tc.tile_pool(name="consts", bufs=1) as consts,
            tc.tile_pool(name="xin", bufs=2) as xin,
            tc.tile_pool(name="oh", bufs=4) as ohp,
            tc.tile_pool(name="head", bufs=1) as headp,
            tc.tile_pool(name="fin", bufs=1) as finp,
        ):
            # --------- small consts + index DMAs (issue first, tiny) ---------
            misc_sb = consts.tile([128, 2 * KT + 6], F32)
            nc.sync.dma_start(misc_sb, misc_d[:, :])
            ind_sb = misc_sb[:, 0:KT]
            lab_sb = misc_sb[:, KT:2 * KT]
            par_sb = misc_sb[:, 2 * KT:2 * KT + 6]
            bias_sb = consts.tile([128, HJ + 1], F32)
            nc.scalar.dma_start(bias_sb, bias_d[:, :])
            bu_sb = bias_sb[:, 0:HJ]
            bp_sb = bias_sb[0:NCLS, HJ:HJ + 1]

            iota_i = consts.tile([128, V], I32)
            nc.gpsimd.iota(iota_i, pattern=[[1, V]], base=0, channel_multiplier=0)
            iota_f = consts.tile([128, V], BF16)
            nc.vector.tensor_copy(iota_f, iota_i)
            iota6_i = consts.tile([128, 6], I32)
            nc.gpsimd.iota(iota6_i, pattern=[[0, 2], [1, 3]], base=0,
                           channel_multiplier=0)
            iota6_f = consts.tile([128, 6], BF16)
            nc.vector.tensor_copy(iota6_f, iota6_i)
            ident = consts.tile([128, 128], F32)
            make_identity(nc, ident)

            # --------- grouped X DMAs (hi on Sync queue, lo on GPSIMD) ---------
            if seg_mode == "hilo":
                xh_g = [xin.tile([128, NG, H], BF16, tag=f"xh{g%2}", name=f"xhg{g}")
                        for g in range(NGRP)]
                xl_g = [xin.tile([128, NG, H], BF16, tag=f"xl{g%2}", name=f"xlg{g}")
                        for g in range(NGRP)]
                for g in range(NGRP):
                    nc.sync.dma_start(
                        xh_g[g], xh_d[g * NG:(g + 1) * NG, :, :]
                        .rearrange("k p h -> p k h"))
                    nc.sync.dma_start(
                        xl_g[g], xl_d[g * NG:(g + 1) * NG, :, :]
                        .rearrange("k p h -> p k h"))
            else:
                xf_g = [xin.tile([128, NG, H], F32R, tag=f"xf{g%2}", name=f"xfg{g}")
                        for g in range(NGRP)]
                for g in range(NGRP):
                    nc.sync.dma_start(
                        xf_g[g], xf_d[g * NG:(g + 1) * NG, :, :]
                        .rearrange("k p h -> p k h"))

            # --------- weight DMAs (Scalar HWDGE queue, after x issues) ---------
            if head_mode == "hilo":
                wuh_sb = consts.tile([128, HJ, H], BF16)
                wul_sb = consts.tile([128, HJ, H], BF16)
                wph_sb = consts.tile([128, HJ, NCLS], BF16)
                wpl_sb = consts.tile([128, HJ, NCLS], BF16)
                nc.sync.dma_start(wuh_sb, wuh_d[:, :, :])
                nc.sync.dma_start(wul_sb, wul_d[:, :, :])
                nc.sync.dma_start(wph_sb, wph_d[:, :, :])
                nc.sync.dma_start(wpl_sb, wpl_d[:, :, :])
            else:
                wuf_sb = consts.tile([128, HJ, H], F32R)
                wpf_sb = consts.tile([128, HJ, NCLS], F32R)
                nc.sync.dma_start(wuf_sb, wuf_d[:, :, :])
                nc.sync.dma_start(wpf_sb, wpf_d[:, :, :])

            # ---------------- phase 1: token loop ----------------
            with tc.tile_pool(name="ps1", bufs=1, space="PSUM") as ps1:
                ps_seg = [ps1.tile([128, V], F32, tag=f"seg{m}", name=f"seg{m}")
                          for m in range(HJ)]
                ps_A = [ps1.tile([128, 6], F32, tag=f"av{s}", name=f"av{s}")
                        for s in range(2)]

                o_dt = BF16 if seg_mode == "hilo" else F32R
                # one-hots + histogram matmuls first: A_v (and the 1/counts
                # broadcast chain) completes while the seg matmuls still run
                O_t = [ohp.tile([128, V], o_dt, tag="O", name=f"O_{k}", bufs=KT)
                       for k in range(KT)]
                for k in range(KT):
                    nc.vector.tensor_scalar(
                        out=O_t[k], in0=iota_f, scalar1=ind_sb[:, k:k + 1],
                        scalar2=None, op0=OP.is_equal)
                    # L6[p,c'] = (lab==c'%3) * parity_mask[p,c']
                    l3_k = ohp.tile([128, 6], BF16, tag="l3", name="l3_k")
                    nc.vector.tensor_scalar(
                        out=l3_k, in0=iota6_f, scalar1=lab_sb[:, k:k + 1],
                        scalar2=None, op0=OP.is_equal)
                    l6_k = ohp.tile([128, 6], o_dt, tag="l6", name="l6_k")
                    nc.vector.tensor_tensor(out=l6_k, in0=l3_k, in1=par_sb,
                                            op=OP.mult)
                    for s in range(2):
                        nc.tensor.matmul(ps_A[s], lhsT=O_t[k][:, s * 128:(s + 1) * 128],
                                         rhs=l6_k, start=(k == 0), stop=(k == KT - 1))

                # -------- stats chain (overlaps the seg loop below) --------
                A_v = [finp.tile([128, 6], F32, tag=f"avsb{s}", name=f"avsb{s}")
                       for s in range(2)]
                recip = [finp.tile([128, 1], F32, tag=f"rc{s}", name=f"rc{s}")
                         for s in range(2)]
                for s in range(2):
                    cnt = finp.tile([128, 1], F32, tag=f"cnt{s}", name=f"cnt{s}")
                    # ACT copy evacuates PSUM and row-sums in one op
                    nc.scalar.activation(A_v[s], ps_A[s], AF.Copy, accum_out=cnt)
                    nc.vector.tensor_scalar_max(cnt, cnt, 1.0)
                    nc.vector.reciprocal(recip[s], cnt)

                # broadcast 1/counts to [128, V] via transpose + K=1 matmul
                # (reuse the released histogram banks av0/av1 of ps1)
                r_row = finp.tile([1, V], F32, tag="rrow")
                for s in range(2):
                    ps_t = ps1.tile([128, 128], F32, tag="av0", name="ps_t")
                    nc.tensor.transpose(ps_t[0:1, 0:128], recip[s], ident)
                    nc.vector.tensor_copy(r_row[0:1, s * 128:(s + 1) * 128],
                                          ps_t[0:1, 0:128])
                ones_row = consts.tile([1, 128], F32)
                nc.vector.memset(ones_row, 1.0)
                ps_bc = ps1.tile([128, V], F32, tag="av1", name="ps_bc")
                nc.tensor.matmul(ps_bc, lhsT=ones_row, rhs=r_row,
                                 start=True, stop=True)
                bc_sb = finp.tile([128, V], F32, tag="bc")
                nc.vector.tensor_copy(bc_sb, ps_bc)

                # -------- seg matmuls --------
                for k in range(KT):
                    g, kk = k // NG, k % NG
                    if seg_mode == "hilo":
                        for m in range(HJ):
                            nc.tensor.matmul(ps_seg[m],
                                             lhsT=xh_g[g][:, kk, m * 128:(m + 1) * 128],
                                             rhs=O_t[k], start=(k == 0), stop=False)
                        for m in range(HJ):
                            nc.tensor.matmul(ps_seg[m],
                                             lhsT=xl_g[g][:, kk, m * 128:(m + 1) * 128],
                                             rhs=O_t[k], start=False,
                                             stop=(k == KT - 1))
                    else:
                        for m in range(HJ):
                            nc.tensor.matmul(ps_seg[m],
                                             lhsT=xf_g[g][:, kk, m * 128:(m + 1) * 128],
                                             rhs=O_t[k],
                                             start=(k == 0), stop=(k == KT - 1))

                # seg_avgT tiles (+ hi/lo split if needed)
                sa_dt = F32 if head_mode == "hilo" else F32R
                sa_f = [headp.tile([128, V], sa_dt, tag=f"saf{m}", name=f"saf{m}")
                        for m in range(HJ)]
                if head_mode == "hilo":
                    sa_h = [headp.tile([128, V], BF16, tag=f"sah{m}", name=f"sah{m}")
                            for m in range(HJ)]
                    sa_l = [headp.tile([128, V], BF16, tag=f"sal{m}", name=f"sal{m}")
                            for m in range(HJ)]
                for m in range(HJ):
                    nc.vector.tensor_tensor(out=sa_f[m], in0=ps_seg[m],
                                            in1=bc_sb, op=OP.mult)
                    if head_mode == "hilo":
                        nc.vector.tensor_copy(sa_h[m], sa_f[m])
                        nc.vector.tensor_tensor(out=sa_l[m], in0=sa_f[m],
                                                in1=sa_h[m], op=OP.subtract)

            # ---------------- phase 3: head ----------------
            th_dt = F32 if head_mode == "hilo" else F32R
            th = [headp.tile([128, V], th_dt, tag=f"th{m}", name=f"th{m}")
                  for m in range(HJ)]
            if head_mode == "hilo":
                hh = [headp.tile([128, V], BF16, tag=f"hh{m}", name=f"hh{m}")
                      for m in range(HJ)]
                hl = [headp.tile([128, V], BF16, tag=f"hl{m}", name=f"hl{m}")
                      for m in range(HJ)]
            with tc.tile_pool(name="ps3", bufs=1, space="PSUM") as ps3:
                for j in range(HJ):
                    ps_h = ps3.tile([128, V], F32, tag="psh", name="ps_h", bufs=2)
                    # kk-outer so PE consumption pipelines with the DVE splits
                    if head_mode == "hilo":
                        n_mm, i_mm = 3 * HJ, 0
                        for kk in range(HJ):
                            for w_sb, x_t in ((wuh_sb, sa_h[kk]), (wuh_sb, sa_l[kk]),
                                              (wul_sb, sa_h[kk])):
                                nc.tensor.matmul(
                                    ps_h, lhsT=w_sb[:, kk, j * 128:(j + 1) * 128],
                                    rhs=x_t, start=(i_mm == 0),
                                    stop=(i_mm == n_mm - 1))
                                i_mm += 1
                    else:
                        for kk in range(HJ):
                            nc.tensor.matmul(
                                ps_h, lhsT=wuf_sb[:, kk, j * 128:(j + 1) * 128],
                                rhs=sa_f[kk], start=(kk == 0),
                                stop=(kk == HJ - 1))
                    # tanh(pre + bu) from PSUM -> SBUF
                    nc.scalar.activation(th[j], ps_h, AF.Tanh,
                                         bias=bu_sb[:, j:j + 1], scale=1.0)
                    if head_mode == "hilo":
                        nc.vector.tensor_copy(hh[j], th[j])
                        nc.vector.tensor_tensor(out=hl[j], in0=th[j], in1=hh[j],
                                                op=OP.subtract)

                # logits
                ps_l = ps3.tile([NCLS, V], F32, tag="psl", name="ps_l")
                if head_mode == "hilo":
                    n_mm, i_mm = 3 * HJ, 0
                    for kk in range(HJ):
                        for w_sb, x_t in ((wph_sb, hh[kk]), (wph_sb, hl[kk]),
                                          (wpl_sb, hh[kk])):
                            nc.tensor.matmul(ps_l, lhsT=w_sb[:, kk, :], rhs=x_t,
                                             start=(i_mm == 0),
                                             stop=(i_mm == n_mm - 1))
                            i_mm += 1
                else:
                    for kk in range(HJ):
                        nc.tensor.matmul(ps_l, lhsT=wpf_sb[:, kk, :],
                                         rhs=th[kk],
                                         start=(kk == 0), stop=(kk == HJ - 1))
                l_sb = finp.tile([NCLS, V], F32, tag="lsb")
                nc.vector.tensor_scalar(out=l_sb, in0=ps_l, scalar1=bp_sb[:, 0:1],
                                        scalar2=None, op0=OP.add)

                # ------------- phase 4: per-segment softmax/argmax -------------
                lv = [finp.tile([128, NCLS], F32, tag=f"lv{s}", name=f"lv{s}")
                      for s in range(2)]
                for s in range(2):
                    ps_t2 = ps3.tile([128, NCLS], F32, tag="pst2", name="ps_t2")
                    nc.tensor.transpose(ps_t2[0:128, 0:NCLS],
                                        l_sb[:, s * 128:(s + 1) * 128],
                                        ident[0:NCLS, 0:NCLS])
                    nc.vector.tensor_copy(lv[s], ps_t2)

                R = [finp.tile([128, 8], F32, tag=f"R{s}", name=f"R{s}")
                     for s in range(2)]
                for s in range(2):
                    ve = nc.vector
                    mneg = finp.tile([128, 1], F32, tag=f"mn{s}", name=f"mn{s}")
                    nc.vector.tensor_reduce(mneg, lv[s],
                                            axis=mybir.AxisListType.X,
                                            op=OP.max, negate=True)
                    e_t = finp.tile([128, NCLS], F32, tag=f"et{s}", name=f"et{s}")
                    ssum = finp.tile([128, 1], F32, tag=f"ss{s}", name=f"ss{s}")
                    nc.scalar.activation(e_t, lv[s], AF.Exp, bias=mneg, scale=1.0,
                                         accum_out=ssum)
                    lse = finp.tile([128, 1], F32, tag=f"lse{s}", name=f"lse{s}")
                    nc.scalar.activation(lse, ssum, AF.Ln)
                    # lse := ln(sum) - mneg = logsumexp
                    ve.tensor_tensor(out=lse, in0=lse, in1=mneg, op=OP.subtract)
                    # cols 4:7 = l - lse = -nll
                    ve.tensor_scalar(out=R[s][:, 4:7], in0=lv[s],
                                     scalar1=lse, scalar2=None, op0=OP.subtract)
                    # eq = (l + mneg) >= 0  (one-hot of rowmax, ties -> multiple)
                    eq = finp.tile([128, NCLS], F32, tag=f"eq{s}", name=f"eq{s}")
                    ve.tensor_scalar(out=eq, in0=lv[s], scalar1=mneg,
                                     scalar2=0.0, op0=OP.add, op1=OP.is_ge)
                    # first-argmax tie-break into cols 0:3
                    ve.tensor_copy(R[s][:, 0:1], eq[:, 0:1])
                    om0 = finp.tile([128, 1], F32, tag=f"om0{s}", name=f"om0{s}")
                    ve.tensor_scalar(out=om0, in0=eq[:, 0:1], scalar1=-1.0,
                                     scalar2=1.0, op0=OP.mult, op1=OP.add)
                    ve.tensor_tensor(out=R[s][:, 1:2], in0=eq[:, 1:2],
                                     in1=om0, op=OP.mult)
                    om1 = finp.tile([128, 1], F32, tag=f"om1{s}", name=f"om1{s}")
                    ve.tensor_scalar(out=om1, in0=eq[:, 1:2], scalar1=-1.0,
                                     scalar2=1.0, op0=OP.mult, op1=OP.add)
                    ve.tensor_tensor(out=om1, in0=om0, in1=om1, op=OP.mult)
                    ve.tensor_tensor(out=R[s][:, 2:3], in0=eq[:, 2:3],
                                     in1=om1, op=OP.mult)
                    nc.gpsimd.memset(R[s][:, 3:4], 1.0)

                # ------------- M2 = A_v^T @ R -------------
                ps_m = ps3.tile([6, 8], F32, tag="psm", name="ps_m")
                for s in range(2):
                    nc.tensor.matmul(ps_m, lhsT=A_v[s], rhs=R[s][:, 0:8],
                                     start=(s == 0), stop=(s == 1))
                m2_sb = finp.tile([6, 8], F32, tag="m2sb")
                nc.vector.tensor_copy(m2_sb, ps_m)
                nc.scalar.dma_start(m2_d[:, :], m2_sb)

    nc.compile()
    return nc


# ---------------------------------------------------------------------------
# host side
# ---------------------------------------------------------------------------
_NC_CACHE = {}


def _get_nc(seg_mode=SEG_MODE, head_mode=HEAD_MODE):
    key = (seg_mode, head_mode)
    if key not in _NC_CACHE:
        _NC_CACHE[key] = build_nc(*key)
    return _NC_CACHE[key]


def _split_hilo(x):
    hi = x.astype(ml_dtypes.bfloat16)
    lo = (x - hi.astype(np.float32)).astype(ml_dtypes.bfloat16)
    return hi, lo


def make_in_maps(encoded_states, indicator, ca_label, Wu, bu, Wp, bp,
                 seg_mode=SEG_MODE, head_mode=HEAD_MODE):
    encoded_states = np.asarray(encoded_states, dtype=np.float32)
    indicator = np.asarray(indicator)
    ca_label = np.asarray(ca_label)
    Wu = np.asarray(Wu, dtype=np.float32)
    Wp = np.asarray(Wp, dtype=np.float32)
    bu = np.asarray(bu, dtype=np.float32)
    bp = np.asarray(bp, dtype=np.float32)

    # shared (weight) tensors
    # SBUF layout [128, HJ, H]: partition p holds WuT[j*128+p, :] per j
    WuT = np.ascontiguousarray(
        Wu.T.reshape(HJ, 128, H).transpose(1, 0, 2))           # [128, HJ, H]
    WpT = np.ascontiguousarray(
        Wp.T.reshape(HJ, 128, NCLS).transpose(1, 0, 2))
    biasp = np.zeros((128, HJ + 1), np.float32)
    biasp[:, 0:HJ] = bu.reshape(HJ, 128).T
    biasp[0:NCLS, HJ] = bp

    shared = {"biasp": biasp}
    if head_mode == "hilo":
        wuh, wul = _split_hilo(WuT)
        wph, wpl = _split_hilo(WpT)
        shared.update({"wut_h": wuh, "wut_l": wul, "wpt_h": wph, "wpt_l": wpl})
    else:
        shared.update({"wut_f": WuT, "wpt_f": WpT})

    in_maps = []
    for b in range(B):
        x = encoded_states[b].reshape(KT, 128, H)
        ind = np.ascontiguousarray(
            indicator[b].astype(np.float32).reshape(KT, 128).T)  # [128,KT]
        lab = np.ascontiguousarray(
            ca_label[b].astype(np.float32).reshape(KT, 128).T)
        miscp = np.zeros((128, 2 * KT + 6), np.float32)
        miscp[:, 0:KT] = ind
        miscp[:, KT:2 * KT] = lab
        miscp[0::2, 2 * KT:2 * KT + 3] = 1.0
        miscp[1::2, 2 * KT + 3:2 * KT + 6] = 1.0
        m = {"miscp": miscp, **shared}
        if seg_mode == "hilo":
            xh, xl = _split_hilo(x)
            m.update({"xh": xh, "xl": xl})
        else:
            m.update({"xf": x})
        in_maps.append(m)
    return in_maps


def assemble(m2_list, loss_weight):
    """Sum per-core M2 [6,8] and finish the scalars exactly as reference."""
    w = np.asarray(loss_weight, dtype=np.float32)
    M = np.zeros((6, 8), np.float64)
    for m2 in m2_list:
        M += m2.astype(np.float64)
    out = []
    for h in range(2):
        Mh = M[3 * h:3 * h + 3, :]
        num = -np.float32(sum(np.float64(w[c]) * Mh[c, 4 + c] for c in range(3)))
        den = np.float32(sum(np.float64(w[c]) * Mh[c, 3] for c in range(3)))
        loss = np.float32(num / den)
        res = {}
        for t in range(3):
            tp = np.int32(round(Mh[t, t]))
            fp = np.int32(round(Mh[:, t].sum() - Mh[t, t]))
            fn = np.int32(round(Mh[t, 3] - Mh[t, t]))
            precision = np.float32(tp) / np.float32(tp + fp)
            recall = np.float32(tp) / np.float32(tp + fn)
            f1 = np.float32(2) * precision * recall / (precision + recall)
            res[t] = {'precision': np.float32(precision),
                      'recall': np.float32(recall), 'f1': np.float32(f1),
                      'tp': tp, 'fp': fp, 'fn': fn}
        out.append((loss, res))
    return ((out[0][0], out[0][1]), (out[1][0], out[1][1]))


def run_device(inputs, seg_mode=SEG_MODE, head_mode=HEAD_MODE, trace=False,
               trace_cores=None):
    nc = _get_nc(seg_mode, head_mode)
    in_maps = make_in_maps(
        inputs["encoded_states"], inputs["indicator"], inputs["ca_label"],
        inputs["Wu"], inputs["bu"], inputs["Wp"], inputs["bp"],
        seg_mode=seg_mode, head_mode=head_mode)
    res = run_bass_kernel_spmd(nc, in_maps, core_ids=list(range(B)),
                               trace=trace, trace_cores=trace_cores)
    m2s = [res.results[c]["m2"] for c in range(B)]
    return m2s, res


def kernel(encoded_states, indicator, ca_label, Wu, bu, Wp, bp, loss_weight):
    inputs = {"encoded_states": encoded_states, "indicator": indicator,
              "ca_label": ca_label, "Wu": Wu, "bu": bu, "Wp": Wp, "bp": bp}
    m2s, _ = run_device(inputs)
    return assemble(m2s, loss_weight)
